# revision 27
# baseline (speedup 1.0000x reference)
"""Causal self-attention (B=4, S=2048, D=1024, single head) on 8 trn2 cores.

v4: 128-row query slots with own-strips-first column permutation. Each core
takes natural query strips {2j+par} at permuted column block j; keys/V are in
the same permuted order (own strips at blocks 0-7, partner strips at 8-15).
Slot j attends permuted key blocks {0..j} u {8..8+j} (capacity 2j+2, total 72
vs 80 score/AV blocks for the 256-row tiling); host masks handle diagonal and
parity-dependent edge blocks. Scores run jointly per slot pair at N=256 over
the pair's shared kc list, plus two N=128 blocks for the odd slot.
"""

import numpy as np
from contextlib import ExitStack

import ml_dtypes

import concourse.bass as bass
import concourse.tile as tile
import concourse.mybir as mybir
from concourse import bacc
from concourse.bass_utils import run_bass_kernel_spmd

F32 = mybir.dt.float32
BF16 = mybir.dt.bfloat16
AFT = mybir.ActivationFunctionType
BF = ml_dtypes.bfloat16

B, S, D = 4, 2048, 1024
P = 128
QTILE = 256
NT = 4
DC = D // P
HK = S // P          # 16 key chunks of 128
SCALE = 1.0 / np.sqrt(D)
MASK_NEG = -1.0e9

_NC_CACHE = None


def _joint_list(u):
    """Shared kc list of slot pair (2u, 2u+1): {0..2u} u {8..8+2u}."""
    return list(range(0, 2 * u + 1)) + list(range(8, 8 + 2 * u + 1))


def _build():
    nc = bacc.Bacc("TRN2", target_bir_lowering=False, debug=False, num_devices=8)
    xt = nc.dram_tensor("XT", [D, S], BF16, kind="ExternalInput").ap()
    wqt = nc.dram_tensor("WqT", [D, D], BF16, kind="ExternalInput").ap()
    wkt = nc.dram_tensor("WkT", [D, D], BF16, kind="ExternalInput").ap()
    wvt = nc.dram_tensor("WvT", [D, D], BF16, kind="ExternalInput").ap()
    mskj = nc.dram_tensor("MaskJ", [4, 2, P, QTILE], F32, kind="ExternalInput").ap()
    mskb = nc.dram_tensor("MaskB", [4, 2, P, P], F32, kind="ExternalInput").ap()
    out = nc.dram_tensor("O", [8 * P, D], F32, kind="ExternalOutput").ap()

    with tile.TileContext(nc) as tc, ExitStack() as ctx:
        persist = ctx.enter_context(tc.tile_pool(name="persist", bufs=1))

        ones_f = persist.tile([P, 2], F32)
        nc.vector.memset(ones_f[:], 1.0)
        ones2 = persist.tile([P, 2], BF16)
        nc.vector.tensor_copy(ones2[:], ones_f[:])
        warm = persist.tile([P, 2], F32)
        nc.scalar.activation(warm[:], ones_f[:], AFT.Exp, scale=1.0)
        wz = persist.tile([P, 512], BF16)
        nc.vector.memset(wz[:], 0.0)

        KT = persist.tile([P, DC, S], BF16)
        V = persist.tile([P, HK, D], BF16)
        QT = persist.tile([P, DC, 8 * P], BF16)
        mtJ = persist.tile([P, 4, 2, QTILE], F32)
        mtB = persist.tile([P, 4, 2, P], F32)



        ev_ctr = [0]

        def evict(dst_ap, src_ap):
            ev_ctr[0] += 1
            if ev_ctr[0] % 2 == 0:
                nc.scalar.copy(dst_ap, src_ap)
            else:
                nc.vector.tensor_copy(dst_ap, src_ap)

        # ---------------- projections ----------------
        # pools managed manually: they stay open through attention pair u=0
        pin_ctx = tc.tile_pool(name="proj_in", bufs=1)
        pin = pin_ctx.__enter__()
        if True:
            XTs = pin.tile([P, DC, S], BF16, tag="xts")
            WkTs = pin.tile([P, DC, D], BF16, tag="wkts")
            WvTs = pin.tile([P, DC, D], BF16, tag="wvts")
            WqTs = pin.tile([P, DC, D], BF16, tag="wqts")
            for dc in range(DC):
                nc.sync.dma_start(XTs[:, dc, 0:S // 2], xt[dc * P:(dc + 1) * P, 0:S // 2])
                nc.sync.dma_start(WkTs[:, dc, :], wkt[dc * P:(dc + 1) * P, :])
            for dc in range(DC):
                nc.sync.dma_start(XTs[:, dc, S // 2:S], xt[dc * P:(dc + 1) * P, S // 2:S])
            for dc in range(DC):
                nc.gpsimd.dma_start(WqTs[:, dc, :], wqt[dc * P:(dc + 1) * P, :])
            for dc in range(DC):
                nc.sync.dma_start(WvTs[:, dc, :], wvt[dc * P:(dc + 1) * P, :])

            kps_ctx = tc.tile_pool(name="proj_ps", bufs=8, space="PSUM")
            kps = kps_ctx.__enter__()
            if True:
                pwu = kps.tile([P, 512], F32, tag="pk", name="pwu")
                for i in range(6):
                    nc.tensor.matmul(pwu[:], wz[:, 0:P], wz[:],
                                     start=(i == 0), stop=(i == 5))
                for wave in range(4):
                    gset = [(4 * (wave // 2) + i, 2 * (wave % 2) + kch)
                            for i in range(4) for kch in range(2)]
                    pks = {g: kps.tile([P, 512], F32, tag="pk",
                                       name=f"pk_{g[0]}_{g[1]}") for g in gset}
                    for dc in range(DC):
                        for g in gset:
                            ec, kch = g
                            nc.tensor.matmul(pks[g][:], WkTs[:, dc, ec * P:(ec + 1) * P],
                                             XTs[:, dc, kch * 512:(kch + 1) * 512],
                                             start=(dc == 0), stop=(dc == DC - 1))
                    for g in gset:
                        ec, kch = g
                        evict(KT[:, ec, kch * 512:(kch + 1) * 512], pks[g][:])

                # mask loads issue here so their HBM traffic stays out of the
                # startup window that gates the K-proj stream
                nc.scalar.dma_start(mtJ[:], mskj.rearrange("u w p j -> p u w j"))
                nc.scalar.dma_start(mtB[:], mskb.rearrange("u w p j -> p u w j"))

                # Q^T: the core's 8 query slots are permuted columns 0..1023
                for ec in range(DC):
                    for qh in range(2):
                        pq = kps.tile([P, 512], F32, tag="pk", name=f"pq_{ec}_{qh}")
                        for dc in range(DC):
                            nc.tensor.matmul(pq[:], WqTs[:, dc, ec * P:(ec + 1) * P],
                                             XTs[:, dc, qh * 512:(qh + 1) * 512],
                                             start=(dc == 0), stop=(dc == DC - 1))
                        evict(QT[:, ec, qh * 512:(qh + 1) * 512], pq[:])
                for kb in range(HK):
                    for eh in range(2):
                        pv = kps.tile([P, 512], F32, tag="pk", name=f"pv_{kb}_{eh}")
                        for dc in range(DC):
                            nc.tensor.matmul(pv[:], XTs[:, dc, kb * P:(kb + 1) * P],
                                             WvTs[:, dc, eh * 512:(eh + 1) * 512],
                                             start=(dc == 0), stop=(dc == DC - 1))
                        evict(V[:, kb, eh * 512:(eh + 1) * 512], pv[:])

        # ---------------- attention ----------------
        def attn_pair(u, mk_ps, mk_pu, mk_pr, pa, pe_pool, po):
            jl = _joint_list(u)
            expS = pe_pool.tile([P, HK, QTILE], BF16, tag="expS",
                                name=f"expS_{u}")
            # joint scores for both slots of the pair, N=256
            for i, m in enumerate(jl):
                pS = mk_ps(f"pS_{u}_{i}")
                for ec in range(DC):
                    nc.tensor.matmul(pS, KT[:, ec, m * P:(m + 1) * P],
                                     QT[:, ec, u * QTILE:(u + 1) * QTILE],
                                     start=(ec == 0), stop=(ec == DC - 1))
                if m == 2 * u:
                    nc.vector.tensor_add(pS, pS, mtJ[:, u, 0, :])
                elif m == 8 + 2 * u:
                    nc.vector.tensor_add(pS, pS, mtJ[:, u, 1, :])
                nc.scalar.activation(expS[:, i, :], pS, AFT.Exp, scale=SCALE)
            # odd-slot-only blocks, N=128
            for w, m in enumerate((2 * u + 1, 9 + 2 * u)):
                pSb = mk_ps(f"pSb_{u}_{w}")
                for ec in range(DC):
                    nc.tensor.matmul(pSb[:, 0:P], KT[:, ec, m * P:(m + 1) * P],
                                     QT[:, ec, u * QTILE + P:(u + 1) * QTILE],
                                     start=(ec == 0), stop=(ec == DC - 1))
                nc.vector.tensor_add(pSb[:, 0:P], pSb[:, 0:P], mtB[:, u, w, :])
                nc.scalar.activation(expS[:, 14 + w, P:QTILE], pSb[:, 0:P],
                                     AFT.Exp, scale=SCALE)
            # AV per slot
            for sl in range(2):
                idxs = list(range(len(jl)))
                if sl == 1:
                    idxs += [14, 15]
                col0 = sl * P
                pU0 = mk_pu(f"pU0_{u}_{sl}")
                pU1 = mk_pu(f"pU1_{u}_{sl}")
                pR = mk_pr(f"pR_{u}_{sl}")
                for ii, i in enumerate(idxs):
                    kc = jl[i] if i < len(jl) else (2 * u + 1 if i == 14 else 9 + 2 * u)
                    lhs = expS[:, i, col0:col0 + P]
                    st, sp = (ii == 0), (ii == len(idxs) - 1)
                    nc.tensor.matmul(pR, lhs, ones2[:], start=st, stop=sp)
                    nc.tensor.matmul(pU0, lhs, V[:, kc, 0:512], start=st, stop=sp)
                    nc.tensor.matmul(pU1, lhs, V[:, kc, 512:1024], start=st, stop=sp)
                rsb = pa.tile([P, 1], F32, tag="rsb", name=f"rsb_{u}_{sl}")
                recip = pa.tile([P, 1], F32, tag="recip", name=f"recip_{u}_{sl}")
                nc.vector.tensor_copy(rsb[:], pR[:, 0:1])
                nc.vector.reciprocal(recip[:], rsb[:])
                ot = po.tile([P, D], F32, tag="ot", name=f"ot_{u}_{sl}")
                row0 = (2 * u + sl) * P
                # normalize+store in quarters, alternating DVE/ACT, so
                # the final store's latency chain is a quarter tile
                for q4 in range(4):
                    lo = q4 * 256
                    src = (pU0 if q4 < 2 else pU1)
                    slo = (q4 % 2) * 256
                    if q4 % 2 == 0:
                        nc.vector.tensor_scalar_mul(ot[:, lo:lo + 256],
                                                    src[:, slo:slo + 256], recip[:])
                    else:
                        nc.scalar.activation(ot[:, lo:lo + 256],
                                             src[:, slo:slo + 256],
                                             AFT.Copy, scale=recip[:])
                    nc.sync.dma_start(out[row0:row0 + P, lo:lo + 256],
                                      ot[:, lo:lo + 256])

        kps_ctx.__exit__(None, None, None)
        pin_ctx.__exit__(None, None, None)
        with tc.tile_pool(name="attn", bufs=2) as pa, \
             tc.tile_pool(name="attn_e", bufs=2) as pe_pool, \
             tc.tile_pool(name="attn_o", bufs=2) as po, \
             tc.tile_pool(name="attn_s", bufs=3, space="PSUM") as psS, \
             tc.tile_pool(name="attn_u", bufs=4, space="PSUM") as psU, \
             tc.tile_pool(name="attn_r", bufs=1, space="PSUM") as psR:
            for u in range(4):
                attn_pair(
                    u,
                    lambda nm: psS.tile([P, QTILE], F32, tag="pS", name=nm)[:],
                    lambda nm: psU.tile([P, 512], F32, tag="pU", name=nm)[:],
                    lambda nm: psR.tile([P, 2], F32, tag="pR", name=nm)[:],
                    pa, pe_pool, po)

    nc.compile()
    return nc


def _get_nc():
    global _NC_CACHE
    if _NC_CACHE is None:
        _NC_CACHE = _build()
    return _NC_CACHE


def _nat_strip(m, parity):
    """Natural 128-strip held at permuted block m (own-first layout)."""
    return 2 * m + parity if m < 8 else 2 * (m - 8) + 1 - parity


def _make_masks(parity):
    """MaskJ [4,2,128,256] for joint blocks {2u, 8+2u} (q = both slots);
    MaskB [4,2,128,128] for odd-slot blocks {2u+1, 9+2u}."""
    mj = np.empty((4, 2, P, QTILE), dtype=np.float32)
    mb = np.empty((4, 2, P, P), dtype=np.float32)
    ki = np.arange(P)[:, None]
    for u in range(4):
        s_a, s_b = 4 * u + parity, 4 * u + 2 + parity
        for w, m in enumerate((2 * u, 8 + 2 * u)):
            kglob = _nat_strip(m, parity) * P + ki
            qa = s_a * P + np.arange(P)[None, :]
            qb = s_b * P + np.arange(P)[None, :]
            mj[u, w, :, 0:P] = np.where(qa >= kglob, 0.0, MASK_NEG)
            mj[u, w, :, P:QTILE] = np.where(qb >= kglob, 0.0, MASK_NEG)
        for w, m in enumerate((2 * u + 1, 9 + 2 * u)):
            kglob = _nat_strip(m, parity) * P + ki
            qb = s_b * P + np.arange(P)[None, :]
            mb[u, w] = np.where(qb >= kglob, 0.0, MASK_NEG)
    return mj, mb


def kernel(X, W_q, W_k, W_v):
    X = np.asarray(X, dtype=np.float32)
    WqT = np.ascontiguousarray(np.asarray(W_q, dtype=np.float32).T).astype(BF)
    WkT = np.ascontiguousarray(np.asarray(W_k, dtype=np.float32).T).astype(BF)
    WvT = np.ascontiguousarray(np.asarray(W_v, dtype=np.float32).T).astype(BF)

    masks = [_make_masks(par) for par in range(2)]
    in_maps = []
    for c in range(8):
        b, par = c // 2, c % 2
        xt = np.asarray(X[b].T)  # [D, S] f32, natural
        cols = np.empty((D, S), dtype=np.float32)
        for m in range(16):
            s = _nat_strip(m, par)
            cols[:, m * P:(m + 1) * P] = xt[:, s * P:(s + 1) * P]
        in_maps.append({
            "XT": cols.astype(BF),
            "WqT": WqT, "WkT": WkT, "WvT": WvT,
            "MaskJ": masks[par][0], "MaskB": masks[par][1],
        })

    global _last_in_maps
    _last_in_maps = in_maps
    nc = _get_nc()
    res = run_bass_kernel_spmd(nc, in_maps, core_ids=list(range(8)))

    out = np.empty((B, S, D), dtype=np.float32)
    for c in range(8):
        b, par = c // 2, c % 2
        oc = res.results[c]["O"]
        for j in range(8):
            s = 2 * j + par
            out[b, s * P:(s + 1) * P, :] = oc[j * P:(j + 1) * P, :]
    return out
